# revision 24
# baseline (speedup 1.0000x reference)
"""ChebNet (K=5, 3 layers) on 8 trn2 NeuronCores.

Sharding: dst-nodes across cores (6250/core, padded to 6272 = 49*128 slots,
degree-sorted within core so per-128-slot-tile max in-degree ~ mean).
Propagation h' = L_hat h is computed as:
    g = dinv * h                           (per-node scale, done where h is made)
    Ptilde[slot] = sum_t g[src(t, slot)]   (dma_gather + identity-matmul
                                            accumulate into PSUM)
    (L_hat h)[slot] = -dinv[slot] * Ptilde (per-partition scale in the
                                            Chebyshev recurrence)
Features are replicated via AllGather of the per-core g slices after each hop.

Gathers use gpsimd.dma_gather (one SWDGE instruction per ~GC edge-tile
columns instead of one per column).  dma_gather indices are int16, so the
50176-row slot space is covered by two overlapping 32768-row windows
(LO=[0,32768), HI=[17408,50176)); each edge picks a window, balanced per
node so per-tile max column counts stay near the mean.
"""
import os
import numpy as np

N_NODES = 50000
N_EDGES = 800000
N_GRAPHS = 64
KCH = 5
C = 128
C_OUT = 16
NCORES = 8
NPC = N_NODES // NCORES          # 6250 real nodes per core
D_TILES = 49
SPC = D_TILES * 128              # 6272 slots per core
NSLOT = NCORES * SPC             # 50176 rows in slot space
P = 128
WIN = 32768                      # int16 index window
HIB = NSLOT - WIN                # 17408: base of the last window
NW = int(os.environ.get("GNN_NW", "6"))       # number of overlapping windows
if NW == 2:
    BASES = [0, HIB]
else:
    BASES = [int(round(HIB / (NW - 1) * i / 128)) * 128
             for i in range(NW - 1)] + [HIB]

GC = int(os.environ.get("GNN_GC", "64"))      # max cols per gather group
QW = int(os.environ.get("GNN_QW", "4"))       # cols accumulated per matmul
_G_BF16 = os.environ.get("GNN_G_BF16", "1") == "1"
GT_BUFS = int(os.environ.get("GNN_GT_BUFS", "6"))   # gather tile ring depth
PS_BUFS = int(os.environ.get("GNN_PS_BUFS", "4"))   # PSUM acc ring depth
NSWQ = int(os.environ.get("GNN_NSWQ", "4"))         # SWDGE queues (max 4)
SP = os.environ.get("GNN_SP", "0") == "1"           # dma_gather single_packet


# ----------------------------------------------------------------------------
# host-side graph preprocessing (index/layout only + norm scalars)
# ----------------------------------------------------------------------------

def prep(x, src, dst, batch):
    src = np.asarray(src).astype(np.int64)
    dst = np.asarray(dst).astype(np.int64)
    x = np.asarray(x, dtype=np.float32)
    batch = np.asarray(batch).astype(np.int64)

    deg = np.bincount(src, minlength=N_NODES).astype(np.float32)
    dinv = np.where(
        deg > 0,
        (1.0 / np.sqrt(np.maximum(deg, 1.0))).astype(np.float32),
        np.float32(0.0),
    ).astype(np.float32)

    indeg = np.bincount(dst, minlength=N_NODES)
    outdeg = np.bincount(src, minlength=N_NODES)

    # slot mapping: snake-deal nodes by in-degree desc across cores (equal
    # per-tile in-degree profiles on every core), and within each block of 8
    # give high-OUT-degree nodes to the middle cores, whose slot rows are
    # covered by the most index windows (more flexible edges -> tighter
    # window split).
    order = np.argsort(-indeg, kind="stable")
    core_pref = [3, 4, 2, 5, 1, 6, 0, 7]
    node_slotrow = np.zeros(N_NODES, np.int64)
    slot_node = np.full((NCORES, SPC), -1, np.int64)
    pos = np.zeros(NCORES, np.int64)
    for blk in range(0, N_NODES, NCORES):
        blknodes = order[blk:blk + NCORES]
        bo = np.argsort(-outdeg[blknodes], kind="stable")
        for rank, ni in enumerate(bo):
            c = core_pref[rank]
            n = blknodes[ni]
            node_slotrow[n] = c * SPC + pos[c]
            slot_node[c, pos[c]] = n
            pos[c] += 1
    assert (pos == NPC).all()

    # per-edge dst-slot decomposition
    srow_d = node_slotrow[dst]              # dst slot row   [E]
    core = srow_d // SPC
    slocal = srow_d % SPC
    dtile = slocal // 128
    lane = slocal % 128
    s = node_slotrow[src]                   # src slot row   [E]

    # ---- NW overlapping int16 windows over the slot space ----
    # window w covers rows [bases[w], bases[w]+WIN); edge feasible-window
    # interval [lo_w, hi_w]; per-tile caps T[w,d] from an exact prefix DP on
    # the interval-Hall constraints, maxed across cores (SPMD uniform).
    feas = np.stack([(s >= b) & (s < b + WIN) for b in BASES])
    lo_w = feas.argmax(0)
    hi_w = NW - 1 - feas[::-1].argmax(0)

    NL = NCORES * 128
    lane_global = core * 128 + lane
    T = np.zeros((NW, D_TILES), np.int64)
    for dd in range(D_TILES):
        m = dtile == dd
        lg, lo_, hi_ = lane_global[m], lo_w[m], hi_w[m]
        cnt = np.zeros((NW, NW, NL), np.int64)
        for a in range(NW):
            for b in range(a, NW):
                mm = (lo_ == a) & (hi_ == b)
                if mm.any():
                    cnt[a, b] = np.bincount(lg[mm], minlength=NL)
        prev = np.zeros(NW + 1, np.int64)
        for b in range(NW):
            best = prev[b]
            for a in range(b + 1):
                m_ab = int(cnt[a:b + 1, a:b + 1].sum(axis=(0, 1)).max())
                best = max(best, prev[a] + m_ab)
            prev[b + 1] = best
        T[:, dd] = np.diff(prev)
    # the PSUM accumulate needs one full-QW-wide run per tile to initialize
    # all QW lanes; bump the widest window if needed
    for dd in range(D_TILES):
        w = int(T[:, dd].argmax())
        if T[w, dd] < QW:
            T[w, dd] = QW
    assert (T.max(axis=0) >= QW).all(), T.max(axis=0)

    # route each edge to a window: per window, earliest-deadline-first
    win_of = np.full(len(s), -1, np.int64)
    for w in range(NW):
        cand = (win_of < 0) & (lo_w <= w) & (w <= hi_w)
        if not cand.any():
            continue
        e = np.flatnonzero(cand)
        order = np.lexsort((hi_w[e], srow_d[e]))
        e = e[order]
        ddv = srow_d[e]
        first = np.r_[True, ddv[1:] != ddv[:-1]]
        starts = np.flatnonzero(first)
        sizes = np.diff(np.r_[starts, len(ddv)])
        rk = np.arange(len(ddv)) - np.repeat(starts, sizes)
        take = rk < T[w][dtile[e]]
        win_of[e[take]] = w
    assert (win_of >= 0).all()

    cumT = np.zeros((NW, D_TILES + 1), np.int64)
    for w in range(NW):
        cumT[w, 1:] = np.cumsum(T[w])
    WB = np.concatenate([[0], np.cumsum(cumT[:, -1])])
    TOT = int(WB[-1])

    # per-window pad slot: a guaranteed-zero row inside the window
    pad_row = []
    for b in BASES:
        pr = None
        for c in range(NCORES):
            r = c * SPC + SPC - 1
            if b <= r < b + WIN:
                pr = r
                break
        assert pr is not None
        pad_row.append(pr)

    # grids in slot space [NCORES, 128, TOT]; window-major column blocks
    grid = np.empty((NCORES, P, TOT), np.int64)
    for w in range(NW):
        grid[:, :, WB[w]:WB[w + 1]] = pad_row[w]
    for w in range(NW):
        mask = win_of == w
        if not mask.any():
            continue
        order = np.argsort(srow_d[mask], kind="stable")
        e = np.flatnonzero(mask)[order]
        ddv = srow_d[e]
        first = np.r_[True, ddv[1:] != ddv[:-1]]
        starts = np.flatnonzero(first)
        sizes = np.diff(np.r_[starts, len(ddv)])
        rk = np.arange(len(ddv)) - np.repeat(starts, sizes)
        assert (rk < T[w][dtile[e]]).all()
        colw = WB[w] + cumT[w][dtile[e]] + rk
        grid[core[e], lane[e], colw] = s[e]

    # device idx: window-relative, int16, 16-partition wrapped, 8x replicated
    gi = grid.copy()
    for w in range(NW):
        gi[:, :, WB[w]:WB[w + 1]] -= BASES[w]
    assert gi.max() < WIN and gi.min() >= 0
    gi16 = gi.astype(np.int16)
    idx_wrapped = np.empty((NCORES, P, 8 * TOT), np.int16)
    for c in range(NCORES):
        w16 = np.transpose(gi16[c].reshape(8, 16, TOT), (1, 2, 0)).reshape(
            16, TOT * 8)
        idx_wrapped[c] = np.tile(w16, (8, 1))

    # per-core [128, 49] tables (lane p, tile d -> slot d*128+p)
    dinv_slot = np.zeros((NCORES, P, D_TILES), np.float32)
    batid_slot = np.full((NCORES, P, D_TILES), 64.0, np.float32)
    x_local = np.zeros((NCORES, SPC, C), np.float32)
    for c in range(NCORES):
        sn = slot_node[c]
        valid = sn >= 0
        dv = np.where(valid, dinv[np.maximum(sn, 0)], 0.0).astype(np.float32)
        bt = np.where(valid, batch[np.maximum(sn, 0)], 64).astype(np.float32)
        dinv_slot[c] = dv.reshape(D_TILES, P).T
        batid_slot[c] = bt.reshape(D_TILES, P).T
        x_local[c][valid] = x[sn[valid]]

    # initial gather source: g0[slotrow] = dinv[n] * x[n]
    g0 = np.zeros((NSLOT, C), np.float32)
    for c in range(NCORES):
        sn = slot_node[c]
        valid = sn >= 0
        g0[c * SPC:(c + 1) * SPC][valid] = (
            x[sn[valid]] * dinv[sn[valid]][:, None]
        )

    cnts = np.bincount(batch, minlength=N_GRAPHS).astype(np.float32)
    inv_cnt = (1.0 / np.maximum(cnts, 1.0)).astype(np.float32)

    iota64 = np.tile(np.arange(64, dtype=np.float32), (P, 1))
    ident = np.eye(P, dtype=np.float32)

    # gather groups: consecutive dst-tiles with <= GC total cols
    tile_cols = T.sum(axis=0)
    groups = []
    da = 0
    while da < D_TILES:
        db = da + 1
        while db < D_TILES and tile_cols[da:db + 1].sum() <= GC:
            db += 1
        groups.append((da, db))
        da = db

    return dict(
        T=T, cumT=cumT, WB=WB, TOT=TOT,
        groups=tuple(groups), grid=grid, idx_wrapped=idx_wrapped,
        dinv_slot=dinv_slot, batid_slot=batid_slot,
        x_local=x_local, g0=g0, inv_cnt=inv_cnt, iota64=iota64, ident=ident,
        slot_node=slot_node, node_slotrow=node_slotrow, dinv=dinv,
    )


# ----------------------------------------------------------------------------
# bass program
# ----------------------------------------------------------------------------

def build(pp, g_bf16):
    import concourse.bacc as bacc
    import concourse.bass as bass
    import concourse.mybir as mybir
    import concourse.tile as tile
    from concourse import library_config

    T, cumT, WB = pp["T"], pp["cumT"], pp["WB"]
    TOT = pp["TOT"]
    groups = pp["groups"]


    f32 = mybir.dt.float32
    i16 = mybir.dt.int16
    g_dt = mybir.dt.bfloat16 if g_bf16 else f32
    AF = mybir.ActivationFunctionType
    OP = mybir.AluOpType

    nc = bacc.Bacc("TRN2", target_bir_lowering=False, debug=False,
                   num_devices=NCORES, num_swdge_queues=NSWQ)

    g0_in = nc.dram_tensor("g0", [NSLOT, C], g_dt, kind="ExternalInput")
    xl_in = nc.dram_tensor("x_local", [SPC, C], g_dt, kind="ExternalInput")
    idx_in = nc.dram_tensor("idxg", [P, 8 * TOT], i16, kind="ExternalInput")
    dinv_in = nc.dram_tensor("dinv", [P, D_TILES], f32, kind="ExternalInput")
    bat_in = nc.dram_tensor("batid", [P, D_TILES], f32, kind="ExternalInput")
    iota_in = nc.dram_tensor("iota64", [P, 64], f32, kind="ExternalInput")
    id_in = nc.dram_tensor("ident", [P, P], f32, kind="ExternalInput")
    w1_in = nc.dram_tensor("W1", [KCH, C, C], f32, kind="ExternalInput")
    w2_in = nc.dram_tensor("W2", [KCH, C, C], f32, kind="ExternalInput")
    w3_in = nc.dram_tensor("W3", [KCH, C, C_OUT], f32, kind="ExternalInput")
    b1_in = nc.dram_tensor("b1", [C, 1], f32, kind="ExternalInput")
    b2_in = nc.dram_tensor("b2", [C, 1], f32, kind="ExternalInput")
    b3r_in = nc.dram_tensor("b3row", [P, C_OUT], f32, kind="ExternalInput")
    ic_in = nc.dram_tensor("inv_cnt", [N_GRAPHS, 1], f32, kind="ExternalInput")
    out_t = nc.dram_tensor("out", [N_GRAPHS, C_OUT], f32, kind="ExternalOutput")

    rg = [list(range(NCORES))]

    with tile.TileContext(nc) as tc:
        with (
            tc.tile_pool(name="const", bufs=1) as cst,
            tc.tile_pool(name="tx", bufs=1) as txp,
            tc.tile_pool(name="gath", bufs=GT_BUFS) as gap,
            tc.tile_pool(name="stg", bufs=4) as stg,
            tc.tile_pool(name="psA", bufs=PS_BUFS, space="PSUM") as psA,
            tc.tile_pool(name="psB", bufs=1, space="PSUM") as psB,
            tc.tile_pool(name="dram", bufs=1, space="DRAM") as drp,
        ):
            nc.gpsimd.load_library(library_config.mlp)

            n_ag = 11
            gbufs = [drp.tile([NSLOT, C], g_dt, addr_space="Shared",
                              name=f"gbuf{i}") for i in range(n_ag)]
            ag_in = drp.tile([SPC, C], g_dt, name="ag_in")
            cc_in = drp.tile([N_GRAPHS, C_OUT], f32, name="cc_in")
            cc_out = drp.tile([N_GRAPHS, C_OUT], f32, addr_space="Shared",
                              name="cc_out")

            idx_sb = cst.tile([P, 8 * TOT], i16, name="idx_sb")
            nc.sync.dma_start(idx_sb[:], idx_in[:])
            dinv = cst.tile([P, D_TILES], f32, name="dinv_sb")
            nc.sync.dma_start(dinv[:], dinv_in[:])
            mdinv = cst.tile([P, D_TILES], f32, name="mdinv_sb")
            nc.vector.tensor_scalar_mul(mdinv[:], dinv[:], -1.0)
            m2dinv = cst.tile([P, D_TILES], f32, name="m2dinv_sb")
            nc.vector.tensor_scalar_mul(m2dinv[:], dinv[:], -2.0)
            batid = cst.tile([P, D_TILES], f32, name="batid_sb")
            nc.sync.dma_start(batid[:], bat_in[:])
            iota64 = cst.tile([P, 64], f32, name="iota64_sb")
            nc.sync.dma_start(iota64[:], iota_in[:])
            identf = cst.tile([P, P], f32, name="identf_sb")
            nc.sync.dma_start(identf[:], id_in[:])
            if g_bf16:
                identg = cst.tile([P, P], g_dt, name="identg_sb")
                nc.vector.tensor_copy(identg[:], identf[:])
            else:
                identg = identf
            w1 = cst.tile([P, KCH * C], f32, name="w1_sb")
            w2 = cst.tile([P, KCH * C], f32, name="w2_sb")
            w3 = cst.tile([P, KCH * C_OUT], f32, name="w3_sb")
            for k in range(KCH):
                nc.sync.dma_start(w1[:, k * C:(k + 1) * C], w1_in[k])
                nc.sync.dma_start(w2[:, k * C:(k + 1) * C], w2_in[k])
                nc.sync.dma_start(w3[:, k * C_OUT:(k + 1) * C_OUT], w3_in[k])
            b1 = cst.tile([C, 1], f32, name="b1_sb")
            nc.sync.dma_start(b1[:], b1_in[:])
            b2 = cst.tile([C, 1], f32, name="b2_sb")
            nc.sync.dma_start(b2[:], b2_in[:])
            b3row = cst.tile([P, C_OUT], f32, name="b3row_sb")
            nc.sync.dma_start(b3row[:], b3r_in[:])
            invc = cst.tile([N_GRAPHS, 1], f32, name="invc_sb")
            nc.sync.dma_start(invc[:], ic_in[:])

            # node-major Chebyshev buffers [128 lanes, 49*128] (lane, d*128+f)
            tx = [txp.tile([P, D_TILES * C], g_dt, name=f"tx{k}_sb")
                  for k in range(KCH)]
            for d in range(D_TILES):
                nc.sync.dma_start(tx[0][:, d * C:(d + 1) * C],
                                  xl_in[d * P:(d + 1) * P, :])

            def do_prop(k, src_dram, layer, dst_buf):
                qload = [0] * NSWQ

                def gather(dst_ap, c0, c1, wbase):
                    ncols = c1 - c0
                    src_ap = src_dram[wbase:wbase + WIN, :]
                    q = min(range(NSWQ), key=lambda i: qload[i])
                    qload[q] += ncols
                    nc.gpsimd.dma_gather(
                        dst_ap, src_ap, idx_sb[:, 8 * c0:8 * c1],
                        128 * ncols, 128 * ncols, C,
                        single_packet=SP,
                        queue_num=q,
                    )

                for gi, (da, db) in enumerate(groups):
                    widths = [int(cumT[w][db] - cumT[w][da]) for w in range(NW)]
                    Wg = sum(widths)
                    gt = gap.tile([P, Wg * C], g_dt, tag="gt", bufs=GT_BUFS)
                    gt3 = gt[:].rearrange("p (a b) -> p a b", a=Wg, b=C)
                    off = 0
                    for w in range(NW):
                        if widths[w] == 0:
                            continue
                        c0 = int(WB[w] + cumT[w][da])
                        c1 = int(WB[w] + cumT[w][db])
                        gather(gt3[:, off:off + widths[w], :], c0, c1, BASES[w])
                        off += widths[w]
                    for d in range(da, db):
                        acc = psA.tile([P, QW * C], f32, tag="acc", bufs=PS_BUFS)
                        runs = []
                        off = 0
                        for w in range(NW):
                            if widths[w] == 0:
                                continue
                            nw_ = int(T[w][d])
                            if nw_:
                                runs.append(
                                    (off + int(cumT[w][d] - cumT[w][da]), nw_))
                            off += widths[w]
                        runs.sort(key=lambda t: -t[1])
                        # QW partial sums in PSUM lanes, folded by a DVE
                        # add tree before the recurrence
                        mms = []
                        for (o, n) in runs:
                            m = 0
                            while m < n:
                                w = min(QW, n - m)
                                mms.append((o + m, w))
                                m += w
                        for i, (o, w) in enumerate(mms):
                            nc.tensor.matmul(
                                acc[:, 0:w * C], identg[:],
                                gt[:, o * C:(o + w) * C],
                                start=(i == 0), stop=(i == len(mms) - 1))
                        pt = stg.tile([P, C], f32, tag="red1")
                        accv = acc[:].rearrange("p (q c) -> p c q", q=QW, c=C)
                        nc.vector.tensor_reduce(
                            pt[:], accv, mybir.AxisListType.X, OP.add)
                        dc = slice(d * C, (d + 1) * C)
                        if k == 1:
                            nc.vector.tensor_scalar(
                                tx[1][:, dc], pt[:], mdinv[:, d:d + 1], None,
                                OP.mult)
                        else:
                            tmp = stg.tile([P, C], g_dt, tag="rtmp")
                            nc.scalar.activation(
                                tmp[:], pt[:], AF.Copy,
                                scale=m2dinv[:, d:d + 1])
                            nc.vector.tensor_tensor(
                                tx[k][:, dc], tmp[:], tx[k - 2][:, dc],
                                OP.subtract)
                        if k <= 3:
                            gs = stg.tile([P, C], g_dt, tag="gs")
                            nc.scalar.activation(
                                gs[:], tx[k][:, dc], AF.Copy,
                                scale=dinv[:, d:d + 1])
                            ag_write(d, gs)
                        else:
                            end_tile(layer, d, dc)
                if dst_buf is not None:
                    do_ag(dst_buf)

            pool_holder = [None]

            def end_tile(layer, d, dc):
                # layer-end transform for one dst tile, interleaved into the
                # k=4 hop (PE transposes, f32 matmuls)
                if layer < 2:
                    wsb = w1 if layer == 0 else w2
                    ops = psB.tile([C, C], f32, tag="wout", bufs=1)
                    for k in range(KCH):
                        txf = stg.tile([P, C], f32, tag="txf")
                        nc.vector.tensor_copy(txf[:], tx[k][:, dc])
                        tp = psB.tile([P, C], f32, tag="tp", bufs=2)
                        nc.tensor.transpose(tp[:], txf[:], identf[:])
                        st = stg.tile([P, C], f32, tag="stgT")
                        nc.vector.tensor_copy(st[:], tp[:])
                        nc.tensor.matmul(
                            ops[:], wsb[:, k * C:(k + 1) * C], st[:],
                            start=(k == 0), stop=(k == KCH - 1))
                    hT = stg.tile([P, C], f32, tag="hT")
                    bsb = b1 if layer == 0 else b2
                    nc.scalar.activation(hT[:], ops[:], AF.Relu, bias=bsb[:])
                    nmp = psB.tile([P, C], f32, tag="tp", bufs=2)
                    nc.tensor.transpose(nmp[:], hT[:], identf[:])
                    nc.vector.tensor_copy(tx[0][:, dc], nmp[:])
                    gs = stg.tile([P, C], g_dt, tag="gs2")
                    nc.vector.tensor_scalar(
                        gs[:], tx[0][:, dc], dinv[:, d:d + 1], None, OP.mult)
                    ag_write(d, gs)
                else:
                    nm3 = psB.tile([P, C_OUT], f32, tag="wout", bufs=1)
                    for k in range(KCH):
                        txf = stg.tile([P, C], f32, tag="txf")
                        nc.vector.tensor_copy(txf[:], tx[k][:, dc])
                        tp = psB.tile([P, C], f32, tag="tp", bufs=2)
                        nc.tensor.transpose(tp[:], txf[:], identf[:])
                        st = stg.tile([P, C], f32, tag="stgT")
                        nc.vector.tensor_copy(st[:], tp[:])
                        nc.tensor.matmul(
                            nm3[:], st[:], w3[:, k * C_OUT:(k + 1) * C_OUT],
                            start=(k == 0), stop=(k == KCH - 1))
                    h3 = stg.tile([P, C_OUT], f32, tag="h3nm")
                    nc.vector.tensor_tensor(h3[:], nm3[:], b3row[:], OP.add)
                    B = stg.tile([P, 64], f32, tag="Bt")
                    nc.vector.tensor_scalar(
                        B[:], iota64[:], batid[:, d:d + 1], None, OP.is_equal)
                    if pool_holder[0] is None:
                        pool_holder[0] = psB.tile([N_GRAPHS, C_OUT], f32,
                                                  tag="pool", bufs=1,
                                                  name="pool_ps")
                    nc.tensor.matmul(pool_holder[0][:], B[:], h3[:],
                                     start=(d == 0), stop=(d == D_TILES - 1))

            def ag_write(d, gs):
                nc.sync.dma_start(ag_in[d * P:(d + 1) * P, :], gs[:])

            def do_ag(dst_buf):
                nc.gpsimd.collective_compute(
                    "AllGather", mybir.AluOpType.bypass,
                    replica_groups=rg,
                    ins=[ag_in.opt()],
                    outs=[dst_buf.opt()],
                )

            ag_i = 0
            cur_src = g0_in
            for layer in range(3):
                for k in range(1, KCH):
                    last = (layer == 2 and k == KCH - 1)
                    dst_buf = None if last else gbufs[ag_i]
                    do_prop(k, cur_src, layer, dst_buf)
                    if dst_buf is not None:
                        cur_src = dst_buf
                        ag_i += 1

            # pooling: partial sums -> AllReduce -> mean -> log_softmax
            pool_sb = stg.tile([N_GRAPHS, C_OUT], f32, name="pool_sb")
            nc.vector.tensor_copy(pool_sb[:], pool_holder[0][:])
            nc.sync.dma_start(cc_in[:], pool_sb[:])
            nc.gpsimd.collective_compute(
                "AllReduce", mybir.AluOpType.add, replica_groups=rg,
                ins=[cc_in.opt()], outs=[cc_out.opt()])
            pooled = stg.tile([N_GRAPHS, C_OUT], f32, name="pooled")
            nc.sync.dma_start(pooled[:], cc_out[:])
            pmean = stg.tile([N_GRAPHS, C_OUT], f32, name="pmean")
            nc.vector.tensor_scalar(pmean[:], pooled[:], invc[:], None, OP.mult)
            mx = stg.tile([N_GRAPHS, 1], f32, name="mx")
            nc.vector.tensor_reduce(mx[:], pmean[:], mybir.AxisListType.X, OP.max)
            z = stg.tile([N_GRAPHS, C_OUT], f32, name="zt")
            nc.vector.tensor_scalar(z[:], pmean[:], mx[:], None, OP.subtract)
            ez = stg.tile([N_GRAPHS, C_OUT], f32, name="ez")
            nc.scalar.activation(ez[:], z[:], AF.Exp)
            sm = stg.tile([N_GRAPHS, 1], f32, name="sm")
            nc.vector.tensor_reduce(sm[:], ez[:], mybir.AxisListType.X, OP.add)
            lg = stg.tile([N_GRAPHS, 1], f32, name="lg")
            nc.scalar.activation(lg[:], sm[:], AF.Ln)
            res = stg.tile([N_GRAPHS, C_OUT], f32, name="res")
            nc.vector.tensor_scalar(res[:], z[:], lg[:], None, OP.subtract)
            nc.sync.dma_start(out_t[:], res[:])

    nc.compile()
    return nc


# ----------------------------------------------------------------------------
# entry point
# ----------------------------------------------------------------------------

_CACHE = {}


def _run(inputs, trace=False):
    from concourse.bass_utils import run_bass_kernel_spmd

    pp = prep(inputs["x"], inputs["src"], inputs["dst"], inputs["batch"])
    key = (int(pp["TOT"]), tuple(pp["T"].ravel()), _G_BF16, GC, NW,
           GT_BUFS, PS_BUFS, NSWQ, SP)
    if key not in _CACHE:
        _CACHE[key] = build(pp, _G_BF16)
    nc = _CACHE[key]

    g0 = pp["g0"]
    x_local = pp["x_local"]
    if _G_BF16:
        import ml_dtypes
        g0 = g0.astype(ml_dtypes.bfloat16)
        x_local = x_local.astype(ml_dtypes.bfloat16)

    b3row = np.tile(np.asarray(inputs["b3"], np.float32).reshape(1, C_OUT),
                    (P, 1))
    in_maps = []
    for c in range(NCORES):
        in_maps.append({
            "g0": g0,
            "x_local": x_local[c],
            "idxg": pp["idx_wrapped"][c],
            "dinv": pp["dinv_slot"][c],
            "batid": pp["batid_slot"][c],
            "iota64": pp["iota64"],
            "ident": pp["ident"],
            "W1": np.asarray(inputs["W1"], np.float32),
            "W2": np.asarray(inputs["W2"], np.float32),
            "W3": np.asarray(inputs["W3"], np.float32),
            "b1": np.asarray(inputs["b1"], np.float32).reshape(C, 1),
            "b2": np.asarray(inputs["b2"], np.float32).reshape(C, 1),
            "b3row": b3row,
            "inv_cnt": pp["inv_cnt"].reshape(N_GRAPHS, 1),
        })
    res = run_bass_kernel_spmd(nc, in_maps, list(range(NCORES)), trace=trace)
    return res.results[0]["out"], res


def kernel(**inputs) -> np.ndarray:
    out, _ = _run(inputs, trace=False)
    return np.asarray(out, dtype=np.float32)

